# revision 24
# baseline (speedup 1.0000x reference)
"""Fused NonLocalBlock2D kernel for Trainium2 (8 NeuronCores, batch-parallel).

Per-core computation (one batch sample, C=64, C2=32, N=64*64=4096):
  xf  = x[b]                          [C, N]
  f   = xf^T xf                       [N, N]   (symmetric, never in HBM)
  p   = softmax(f, axis=-1)
  gx  = xf^T g_w^T                    [N, C2]
  y   = p gx                          [N, C2]
  out = W_w y^T + W_b + xf            [C, N]

Design notes (v3 — ACT-exp is the roofline: 16.7M exps at 1
elem/cycle/lane @1.2GHz = 109us; PE needs ~110us @2.4GHz):
  - Host precomputes everything outside the N^2 stream: the gx
    projection (stationary of the second pass, 33rd ones column makes
    y0 row 32 the softmax denominator), the residual base
    xb = x + (W_w g_b + W_b) (g_b folds out: softmax rows sum to 1),
    and a per-sample constant softmax shift c = max_n ||x_n||^2 - 20
    fed as the exp ACTIVATE's per-partition bias. A constant shift is
    exact (cancels in num/den); c bounds scores via Cauchy-Schwarz so
    exp <= e^20, and the smallest denominator stays normal fp32.
    The shift-as-bias removes the K=65 fused -D row: S is a pure K=64
    xf^T xf matmul.
  - No Ln on ACT: 1/denominator via DVE reciprocal -> ACT runs Exp
    only -> one activation-table load, no swap stalls (baseline lost
    ~14us to 9 ACT_TABLE_LOADs).
  - PSUM: 3 s-buffers [128,1024] (6 banks) + y0 [33,1024] (2 banks).
    The quarter tail borrows one s-slot for its z / rbc matmul
    outputs instead of dedicated banks.
  - f32r operands come straight from DMA'd fp32 bits via .bitcast
    (PE f32r mode: 1 cycle/row when moving free >= 256); exp output
    is written as f32r by ACT.
"""

import numpy as np

_REPO = "/opt/trn_rl_repo"

C = 64
C2 = 32
N = 4096
MC = 128          # m-chunk width (partition dim of E tiles)
NMC = N // MC     # 32 m-chunks
QW = 1024         # n-quarter width (PSUM: 2 banks)
NQ = N // QW      # 4 quarters
HB = 512          # half-quarter / psum-bank width

_CACHE = {}


def _ensure_path():
    import sys
    if _REPO not in sys.path:
        sys.path.insert(0, _REPO)


def _build_nc():
    _ensure_path()
    import concourse.tile as tile
    from concourse import bacc, mybir
    from contextlib import ExitStack

    fp32 = mybir.dt.float32
    f32r = mybir.dt.float32r
    AF = mybir.ActivationFunctionType

    nc = bacc.Bacc(
        "TRN2",
        target_bir_lowering=False,
        debug=False,
        enable_asserts=True,
        num_devices=8,
    )

    xfo_d = nc.dram_tensor("xfo65", [C + 1, N], fp32, kind="ExternalInput").ap()
    xfd_d = nc.dram_tensor("xfd65", [C + 1, N], fp32, kind="ExternalInput").ap()
    gx_d = nc.dram_tensor("gx33", [MC, 33 * NMC], fp32, kind="ExternalInput").ap()
    WwT_d = nc.dram_tensor("W_wT", [C2, C], fp32, kind="ExternalInput").ap()
    xb_d = nc.dram_tensor("xb", [C, N], fp32, kind="ExternalInput").ap()
    out_d = nc.dram_tensor("out", [C, N], fp32, kind="ExternalOutput").ap()

    with tile.TileContext(nc) as tc, ExitStack() as ctx:
        persist = ctx.enter_context(tc.tile_pool(name="persist", bufs=1))
        xfo = persist.tile([C + 1, N], fp32)     # rows 0-63 xf, row 64 = 1.0
        xfoR = persist.tile([C + 1, N], f32r)    # S stationary
        xfd = persist.tile([C + 1, N], fp32)     # rows 0-63 xf, row 64 = -D
        xfdR = persist.tile([C + 1, N], f32r)    # S moving
        gxs = persist.tile([MC, 33 * NMC], fp32)
        gxr = persist.tile([MC, 33 * NMC], f32r)
        WwT_f = persist.tile([C2, C], fp32)
        WwT_r = persist.tile([C2, C], f32r)
        xbt = persist.tile([C, N], fp32)
        ones1 = persist.tile([C2 + 1, C2], fp32)    # row 32 used (lane-aligned w/ d)
        ones1r = persist.tile([C2 + 1, C2], f32r)

        # Two HWDGE queues (sync + scalar) for the two big streams, chunked
        # 4x so the f32r converts (and then the first S matmuls) start as
        # soon as the first 1024 columns land; gx/WwT ride the GpSimd
        # SWDGE queue. xb is only needed at the first quarter tail -> last.
        for h in range(4):
            sl = slice(h * QW, (h + 1) * QW)
            nc.sync.dma_start(xfo[:, sl], xfo_d[:, sl])
            nc.scalar.dma_start(xfd[:, sl], xfd_d[:, sl])
            nc.scalar.activation(xfoR[:, sl], xfo[:, sl], AF.Copy)
            nc.vector.tensor_copy(xfdR[:, sl], xfd[:, sl])
        nc.gpsimd.dma_start(gxs[:], gx_d)
        nc.gpsimd.dma_start(WwT_f[:], WwT_d)
        nc.sync.dma_start(xbt[:], xb_d)
        nc.any.memset(ones1[C2 : C2 + 1, :], 1.0)

        nc.vector.tensor_copy(gxr[:], gxs[:])
        nc.vector.tensor_copy(WwT_r[:], WwT_f[:])
        nc.vector.tensor_copy(ones1r[C2 : C2 + 1, :], ones1[C2 : C2 + 1, :])

        gxR = gxr[:]
        WwT_R = WwT_r[:]

        # Manual 3-slot carve of 6 PSUM banks: one [128, 3072] tile, slot i
        # = columns [i*1024, (i+1)*1024). Adjacent-slot chunk pairs get ONE
        # fused [128, 2048] exp (TRN2 ACTIVATE can read across PSUM banks),
        # halving the per-instruction ACT overhead. Subtile dep tracking
        # orders the slot reuse.
        sp_pool = ctx.enter_context(tc.tile_pool(name="spsum", bufs=1, space="PSUM"))
        s3 = sp_pool.tile([MC, 3 * QW], fp32)
        y_pool = ctx.enter_context(tc.tile_pool(name="ypsum", bufs=1, space="PSUM"))
        y0 = y_pool.tile([C2 + 1, QW], fp32)     # banks 6-7, partitions 0-32

        e_pool = ctx.enter_context(tc.tile_pool(name="e", bufs=4))
        ysb_pool = ctx.enter_context(tc.tile_pool(name="ysb", bufs=2))
        y1_pool = ctx.enter_context(tc.tile_pool(name="y1", bufs=2))
        r_pool = ctx.enter_context(tc.tile_pool(name="r", bufs=2))
        o_pool = ctx.enter_context(tc.tile_pool(name="osb", bufs=3))

        slot_state = [0]

        def next_slot():
            s = slot_state[0]
            slot_state[0] = (s + 1) % 3
            return s

        def tail_part2(n0, r_f, y_sb):
            # previous quarter's normalize + W-projection + residual + store.
            # Emitted mid-way through the NEXT quarter so the r-dependent
            # matmuls never head-of-line-block the PE FIFO. Borrows the next
            # s3 slot for its PSUM outputs.
            sc = next_slot() * QW
            rbc = s3[0:C2, sc + HB : sc + QW]
            z_t = s3[0:C, sc : sc + HB]
            for h in range(2):
                nc.tensor.matmul(
                    rbc,
                    lhsT=ones1r[C2 : C2 + 1, :],
                    rhs=r_f[:, h * HB : (h + 1) * HB],
                    start=True,
                    stop=True,
                )
                y1 = y1_pool.tile([C2, HB], f32r)
                nc.vector.tensor_mul(y1[:], y_sb[:, h * HB : (h + 1) * HB], rbc)
                nc.tensor.matmul(
                    z_t,
                    lhsT=WwT_R[:],
                    rhs=y1[:],
                    start=True,
                    stop=True,
                )
                o_t = o_pool.tile([C, HB], fp32)
                nc.vector.tensor_add(
                    o_t[:], z_t, xbt[:, n0 + h * HB : n0 + (h + 1) * HB]
                )
                nc.sync.dma_start(
                    out_d[:, n0 + h * HB : n0 + (h + 1) * HB], o_t[:]
                )

        pending = None
        for nq in range(NQ):
            n0 = nq * QW
            for j in range(NMC // 2):
                qa, qb = 2 * j, 2 * j + 1
                sa, sb = next_slot(), next_slot()
                for q, sl in ((qa, sa), (qb, sb)):
                    for h in range(2):
                        nc.tensor.matmul(
                            s3[:, sl * QW + h * HB : sl * QW + (h + 1) * HB],
                            lhsT=xfoR[:, q * MC : (q + 1) * MC],
                            rhs=xfdR[:, n0 + h * HB : n0 + (h + 1) * HB],
                            start=True,
                            stop=True,
                        )
                e_t = e_pool.tile([MC, 2 * QW], f32r)
                if sb == sa + 1:
                    nc.scalar.activation(
                        e_t[:], s3[:, sa * QW : sa * QW + 2 * QW], AF.Exp
                    )
                else:
                    nc.scalar.activation(
                        e_t[:, 0:QW], s3[:, sa * QW : (sa + 1) * QW], AF.Exp
                    )
                    nc.scalar.activation(
                        e_t[:, QW : 2 * QW], s3[:, sb * QW : (sb + 1) * QW], AF.Exp
                    )
                for q, off in ((qa, 0), (qb, QW)):
                    for h in range(2):
                        nc.tensor.matmul(
                            y0[:, h * HB : (h + 1) * HB],
                            lhsT=gxR[:, q * 33 : (q + 1) * 33],
                            rhs=e_t[:, off + h * HB : off + (h + 1) * HB],
                            start=(q == 0),
                            stop=(q == NMC - 1),
                        )
                if j == 5 and pending is not None:
                    tail_part2(*pending)
                    pending = None

            # tail part 1: evacuate y0 (numerators + denominator row) to
            # SBUF in ONE copy -- the next quarter's y0 accumulation only
            # waits on this; the reciprocal then runs off the critical
            # path from the SBUF copy. For the last quarter ACT computes
            # r = exp(-ln(d)) instead: ACT is idle after the final exp
            # tile and its ~2us (+2 table swaps) beats the DVE iterative
            # divide's 6.5us on the exposed final tail.
            y_sb33 = ysb_pool.tile([C2 + 1, QW], fp32)
            nc.vector.tensor_copy(y_sb33[:], y0[:])
            y_sb = y_sb33[0:C2, :]
            d_sb = y_sb33[C2 : C2 + 1, :]        # partition 32, lane-aligned
            r_t = r_pool.tile([C2 + 1, QW], f32r, tag="rr")
            r_f = r_t[C2 : C2 + 1, :]
            if nq == NQ - 1:
                ln_t = r_pool.tile([C2 + 1, QW], fp32, tag="ln")
                nc.scalar.activation(ln_t[C2 : C2 + 1, :], d_sb, AF.Ln)
                nc.scalar.activation(
                    r_f, ln_t[C2 : C2 + 1, :], AF.Exp, scale=-1.0
                )
            else:
                with nc.allow_low_precision(reason="1/d f32r; 11-bit ok"):
                    nc.vector.reciprocal(r_f, d_sb)
            pending = (n0, r_f, y_sb)
        tail_part2(*pending)

    nc.compile()
    return nc


def _get_nc():
    if "nc" not in _CACHE:
        _CACHE["nc"] = _build_nc()
    return _CACHE["nc"]


def _run(inputs, trace=False, **kw):
    _ensure_path()
    from concourse.bass_utils import run_bass_kernel_spmd

    nc = _get_nc()
    x = np.ascontiguousarray(np.asarray(inputs["x"], dtype=np.float32))
    g_w = np.asarray(inputs["g_w"], dtype=np.float32)
    g_b = np.asarray(inputs["g_b"], dtype=np.float32)
    W_w = np.asarray(inputs["W_w"], dtype=np.float32)
    W_b = np.asarray(inputs["W_b"], dtype=np.float32)

    WwT = np.ascontiguousarray(W_w.T)                         # [C2, C]
    b_eff = (
        W_w.astype(np.float64) @ g_b.astype(np.float64) + W_b.astype(np.float64)
    ).astype(np.float32).reshape(C, 1)

    B = x.shape[0]
    in_maps = []
    for i in range(B):
        xf = np.ascontiguousarray(x[i].reshape(C, N))
        D = np.einsum("cn,cn->n", xf, xf).astype(np.float32)
        gx = xf.T @ g_w.T                                     # [N, C2]
        gx33 = np.concatenate([gx, np.ones((N, 1), np.float32)], axis=1)
        gx33 = np.ascontiguousarray(
            gx33.reshape(NMC, MC, 33).transpose(1, 0, 2).reshape(MC, 33 * NMC)
        )
        in_maps.append(
            {
                "xfo65": np.ascontiguousarray(
                    np.concatenate([xf, np.ones((1, N), np.float32)], axis=0)
                ),
                "xfd65": np.ascontiguousarray(
                    np.concatenate([xf, -D[None, :]], axis=0)
                ),
                "gx33": gx33,
                "W_wT": WwT,
                "xb": np.ascontiguousarray(xf + b_eff),
            }
        )
    res = run_bass_kernel_spmd(nc, in_maps, list(range(B)), trace=trace, **kw)
    out = np.stack([res.results[i]["out"].reshape(C, 64, 64) for i in range(B)])
    return res, out.astype(np.float32)


def kernel(**inputs):
    _, out = _run(inputs, trace=False)
    return out


# revision 25
# speedup vs baseline: 2.3112x; 2.3112x over previous
"""Fused NonLocalBlock2D kernel for Trainium2 (8 NeuronCores, batch-parallel).

Per-core computation (one batch sample, C=64, C2=32, N=64*64=4096):
  xf  = x[b]                          [C, N]
  f   = xf^T xf                       [N, N]   (symmetric, never in HBM)
  p   = softmax(f, axis=-1)
  gx  = xf^T g_w^T                    [N, C2]
  y   = p gx                          [N, C2]
  out = W_w y^T + W_b + xf            [C, N]

Design notes (v3 — ACT-exp is the roofline: 16.7M exps at 1
elem/cycle/lane @1.2GHz = 109us; PE needs ~110us @2.4GHz):
  - Host precomputes everything outside the N^2 stream: the gx
    projection (stationary of the second pass, 33rd ones column makes
    y0 row 32 the softmax denominator), the residual base
    xb = x + (W_w g_b + W_b) (g_b folds out: softmax rows sum to 1),
    and a per-sample constant softmax shift c = max_n ||x_n||^2 - 20
    fed as the exp ACTIVATE's per-partition bias. A constant shift is
    exact (cancels in num/den); c bounds scores via Cauchy-Schwarz so
    exp <= e^20, and the smallest denominator stays normal fp32.
    The shift-as-bias removes the K=65 fused -D row: S is a pure K=64
    xf^T xf matmul.
  - No Ln on ACT: 1/denominator via DVE reciprocal -> ACT runs Exp
    only -> one activation-table load, no swap stalls (baseline lost
    ~14us to 9 ACT_TABLE_LOADs).
  - PSUM: 3 s-buffers [128,1024] (6 banks) + y0 [33,1024] (2 banks).
    The quarter tail borrows one s-slot for its z / rbc matmul
    outputs instead of dedicated banks.
  - f32r operands come straight from DMA'd fp32 bits via .bitcast
    (PE f32r mode: 1 cycle/row when moving free >= 256); exp output
    is written as f32r by ACT.
"""

import numpy as np

_REPO = "/opt/trn_rl_repo"

C = 64
C2 = 32
N = 4096
MC = 128          # m-chunk width (partition dim of E tiles)
NMC = N // MC     # 32 m-chunks
QW = 1024         # n-quarter width (PSUM: 2 banks)
NQ = N // QW      # 4 quarters
HB = 512          # half-quarter / psum-bank width

_CACHE = {}


def _ensure_path():
    import sys
    if _REPO not in sys.path:
        sys.path.insert(0, _REPO)


def _build_nc():
    _ensure_path()
    import concourse.tile as tile
    from concourse import bacc, mybir
    from contextlib import ExitStack

    fp32 = mybir.dt.float32
    f32r = mybir.dt.float32r
    AF = mybir.ActivationFunctionType

    nc = bacc.Bacc(
        "TRN2",
        target_bir_lowering=False,
        debug=False,
        enable_asserts=True,
        num_devices=8,
    )

    xfo_d = nc.dram_tensor("xfo65", [C + 1, N], fp32, kind="ExternalInput").ap()
    xfd_d = nc.dram_tensor("xfd65", [C + 1, N], fp32, kind="ExternalInput").ap()
    gx_d = nc.dram_tensor("gx33", [MC, 33 * NMC], fp32, kind="ExternalInput").ap()
    WwT_d = nc.dram_tensor("W_wT", [C2, C], fp32, kind="ExternalInput").ap()
    xb_d = nc.dram_tensor("xb", [C, N], fp32, kind="ExternalInput").ap()
    out_d = nc.dram_tensor("out", [C, N], fp32, kind="ExternalOutput").ap()

    with tile.TileContext(nc) as tc, ExitStack() as ctx:
        persist = ctx.enter_context(tc.tile_pool(name="persist", bufs=1))
        xfo = persist.tile([C + 1, N], fp32)     # rows 0-63 xf, row 64 = 1.0
        xfoR = persist.tile([C + 1, N], f32r)    # S stationary
        xfd = persist.tile([C + 1, N], fp32)     # rows 0-63 xf, row 64 = -D
        xfdR = persist.tile([C + 1, N], f32r)    # S moving
        gxs = persist.tile([MC, 33 * NMC], fp32)
        gxr = persist.tile([MC, 33 * NMC], f32r)
        WwT_f = persist.tile([C2, C], fp32)
        WwT_r = persist.tile([C2, C], f32r)
        xbt = persist.tile([C, N], fp32)
        ones1 = persist.tile([C2 + 1, C2], fp32)    # row 32 used (lane-aligned w/ d)
        ones1r = persist.tile([C2 + 1, C2], f32r)

        # Two HWDGE queues (sync + scalar) for the two big streams, chunked
        # 4x so the f32r converts (and then the first S matmuls) start as
        # soon as the first 1024 columns land; gx/WwT ride the GpSimd
        # SWDGE queue. xb is only needed at the first quarter tail -> last.
        for h in range(4):
            sl = slice(h * QW, (h + 1) * QW)
            nc.sync.dma_start(xfo[:, sl], xfo_d[:, sl])
            nc.scalar.dma_start(xfd[:, sl], xfd_d[:, sl])
            nc.scalar.activation(xfoR[:, sl], xfo[:, sl], AF.Copy)
            nc.vector.tensor_copy(xfdR[:, sl], xfd[:, sl])
        nc.gpsimd.dma_start(gxs[:], gx_d)
        nc.gpsimd.dma_start(WwT_f[:], WwT_d)
        nc.sync.dma_start(xbt[:], xb_d)
        nc.any.memset(ones1[C2 : C2 + 1, :], 1.0)

        nc.vector.tensor_copy(gxr[:], gxs[:])
        nc.vector.tensor_copy(WwT_r[:], WwT_f[:])
        nc.vector.tensor_copy(ones1r[C2 : C2 + 1, :], ones1[C2 : C2 + 1, :])

        gxR = gxr[:]
        WwT_R = WwT_r[:]

        s_pool = ctx.enter_context(tc.tile_pool(name="spsum", bufs=3, space="PSUM"))
        y_pool = ctx.enter_context(tc.tile_pool(name="ypsum", bufs=1, space="PSUM"))
        y0 = y_pool.tile([C2 + 1, QW], fp32)     # banks 6-7, partitions 0-32

        e_pool = ctx.enter_context(tc.tile_pool(name="e", bufs=7))
        ysb_pool = ctx.enter_context(tc.tile_pool(name="ysb", bufs=2))
        y1_pool = ctx.enter_context(tc.tile_pool(name="y1", bufs=2))
        r_pool = ctx.enter_context(tc.tile_pool(name="r", bufs=2))
        o_pool = ctx.enter_context(tc.tile_pool(name="osb", bufs=3))

        def tail_part2(n0, r_f, y_sb):
            # previous quarter's normalize + W-projection + residual + store.
            # Emitted mid-way through the NEXT quarter so the r-dependent
            # matmuls never head-of-line-block the PE FIFO. Borrows one
            # s-slot for its PSUM outputs.
            borrow = s_pool.tile([MC, QW], fp32, tag="S")
            rbc = borrow[0:C2, HB:QW]
            z_t = borrow[0:C, 0:HB]
            for h in range(2):
                nc.tensor.matmul(
                    rbc,
                    lhsT=ones1r[C2 : C2 + 1, :],
                    rhs=r_f[:, h * HB : (h + 1) * HB],
                    start=True,
                    stop=True,
                )
                y1 = y1_pool.tile([C2, HB], f32r)
                nc.vector.tensor_mul(y1[:], y_sb[:, h * HB : (h + 1) * HB], rbc)
                nc.tensor.matmul(
                    z_t,
                    lhsT=WwT_R[:],
                    rhs=y1[:],
                    start=True,
                    stop=True,
                )
                o_t = o_pool.tile([C, HB], fp32)
                nc.vector.tensor_add(
                    o_t[:], z_t, xbt[:, n0 + h * HB : n0 + (h + 1) * HB]
                )
                nc.sync.dma_start(
                    out_d[:, n0 + h * HB : n0 + (h + 1) * HB], o_t[:]
                )

        pending = None
        for nq in range(NQ):
            n0 = nq * QW
            for q in range(NMC):
                s_t = s_pool.tile([MC, QW], fp32, tag="S")
                for h in range(2):
                    nc.tensor.matmul(
                        s_t[:, h * HB : (h + 1) * HB],
                        lhsT=xfoR[:, q * MC : (q + 1) * MC],
                        rhs=xfdR[:, n0 + h * HB : n0 + (h + 1) * HB],
                        start=True,
                        stop=True,
                    )
                e_t = e_pool.tile([MC, QW], f32r)
                nc.scalar.activation(e_t[:], s_t[:], AF.Exp)
                for h in range(2):
                    nc.tensor.matmul(
                        y0[:, h * HB : (h + 1) * HB],
                        lhsT=gxR[:, q * 33 : (q + 1) * 33],
                        rhs=e_t[:, h * HB : (h + 1) * HB],
                        start=(q == 0),
                        stop=(q == NMC - 1),
                    )
                if q == 10 and pending is not None:
                    tail_part2(*pending)
                    pending = None

            # tail part 1: evacuate y0 (numerators + denominator row) to
            # SBUF in ONE copy -- the next quarter's y0 accumulation only
            # waits on this; the reciprocal then runs off the critical
            # path from the SBUF copy. For the last quarter ACT computes
            # r = exp(-ln(d)) instead: ACT is idle after the final exp
            # tile and its ~2us (+2 table swaps) beats the DVE iterative
            # divide's 6.5us on the exposed final tail.
            y_sb33 = ysb_pool.tile([C2 + 1, QW], fp32)
            nc.vector.tensor_copy(y_sb33[:], y0[:])
            y_sb = y_sb33[0:C2, :]
            d_sb = y_sb33[C2 : C2 + 1, :]        # partition 32, lane-aligned
            r_t = r_pool.tile([C2 + 1, QW], f32r, tag="rr")
            r_f = r_t[C2 : C2 + 1, :]
            if nq == NQ - 1:
                ln_t = r_pool.tile([C2 + 1, QW], fp32, tag="ln")
                nc.scalar.activation(ln_t[C2 : C2 + 1, :], d_sb, AF.Ln)
                nc.scalar.activation(
                    r_f, ln_t[C2 : C2 + 1, :], AF.Exp, scale=-1.0
                )
            else:
                with nc.allow_low_precision(reason="1/d f32r; 11-bit ok"):
                    nc.vector.reciprocal(r_f, d_sb)
            pending = (n0, r_f, y_sb)
        tail_part2(*pending)

    nc.compile()
    return nc


def _get_nc():
    if "nc" not in _CACHE:
        _CACHE["nc"] = _build_nc()
    return _CACHE["nc"]


def _run(inputs, trace=False, **kw):
    _ensure_path()
    from concourse.bass_utils import run_bass_kernel_spmd

    nc = _get_nc()
    x = np.ascontiguousarray(np.asarray(inputs["x"], dtype=np.float32))
    g_w = np.asarray(inputs["g_w"], dtype=np.float32)
    g_b = np.asarray(inputs["g_b"], dtype=np.float32)
    W_w = np.asarray(inputs["W_w"], dtype=np.float32)
    W_b = np.asarray(inputs["W_b"], dtype=np.float32)

    WwT = np.ascontiguousarray(W_w.T)                         # [C2, C]
    b_eff = (
        W_w.astype(np.float64) @ g_b.astype(np.float64) + W_b.astype(np.float64)
    ).astype(np.float32).reshape(C, 1)

    B = x.shape[0]
    in_maps = []
    for i in range(B):
        xf = np.ascontiguousarray(x[i].reshape(C, N))
        D = np.einsum("cn,cn->n", xf, xf).astype(np.float32)
        gx = xf.T @ g_w.T                                     # [N, C2]
        gx33 = np.concatenate([gx, np.ones((N, 1), np.float32)], axis=1)
        gx33 = np.ascontiguousarray(
            gx33.reshape(NMC, MC, 33).transpose(1, 0, 2).reshape(MC, 33 * NMC)
        )
        in_maps.append(
            {
                "xfo65": np.ascontiguousarray(
                    np.concatenate([xf, np.ones((1, N), np.float32)], axis=0)
                ),
                "xfd65": np.ascontiguousarray(
                    np.concatenate([xf, -D[None, :]], axis=0)
                ),
                "gx33": gx33,
                "W_wT": WwT,
                "xb": np.ascontiguousarray(xf + b_eff),
            }
        )
    res = run_bass_kernel_spmd(nc, in_maps, list(range(B)), trace=trace, **kw)
    out = np.stack([res.results[i]["out"].reshape(C, 64, 64) for i in range(B)])
    return res, out.astype(np.float32)


def kernel(**inputs):
    _, out = _run(inputs, trace=False)
    return out


# revision 26
# speedup vs baseline: 2.3209x; 1.0042x over previous
"""Fused NonLocalBlock2D kernel for Trainium2 (8 NeuronCores, batch-parallel).

Per-core computation (one batch sample, C=64, C2=32, N=64*64=4096):
  xf  = x[b]                          [C, N]
  f   = xf^T xf                       [N, N]   (symmetric, never in HBM)
  p   = softmax(f, axis=-1)
  gx  = xf^T g_w^T                    [N, C2]
  y   = p gx                          [N, C2]
  out = W_w y^T + W_b + xf            [C, N]

Design notes (v3 — ACT-exp is the roofline: 16.7M exps at 1
elem/cycle/lane @1.2GHz = 109us; PE needs ~110us @2.4GHz):
  - Host precomputes everything outside the N^2 stream: the gx
    projection (stationary of the second pass, 33rd ones column makes
    y0 row 32 the softmax denominator), the residual base
    xb = x + (W_w g_b + W_b) (g_b folds out: softmax rows sum to 1),
    and a per-sample constant softmax shift c = max_n ||x_n||^2 - 20
    fed as the exp ACTIVATE's per-partition bias. A constant shift is
    exact (cancels in num/den); c bounds scores via Cauchy-Schwarz so
    exp <= e^20, and the smallest denominator stays normal fp32.
    The shift-as-bias removes the K=65 fused -D row: S is a pure K=64
    xf^T xf matmul.
  - No Ln on ACT: 1/denominator via DVE reciprocal -> ACT runs Exp
    only -> one activation-table load, no swap stalls (baseline lost
    ~14us to 9 ACT_TABLE_LOADs).
  - PSUM: 3 s-buffers [128,1024] (6 banks) + y0 [33,1024] (2 banks).
    The quarter tail borrows one s-slot for its z / rbc matmul
    outputs instead of dedicated banks.
  - f32r operands come straight from DMA'd fp32 bits via .bitcast
    (PE f32r mode: 1 cycle/row when moving free >= 256); exp output
    is written as f32r by ACT.
"""

import numpy as np

_REPO = "/opt/trn_rl_repo"

C = 64
C2 = 32
N = 4096
MC = 128          # m-chunk width (partition dim of E tiles)
NMC = N // MC     # 32 m-chunks
QW = 1024         # n-quarter width (PSUM: 2 banks)
NQ = N // QW      # 4 quarters
HB = 512          # half-quarter / psum-bank width

_CACHE = {}


def _ensure_path():
    import sys
    if _REPO not in sys.path:
        sys.path.insert(0, _REPO)


def _build_nc():
    _ensure_path()
    import concourse.tile as tile
    from concourse import bacc, mybir
    from contextlib import ExitStack

    fp32 = mybir.dt.float32
    f32r = mybir.dt.float32r
    AF = mybir.ActivationFunctionType

    nc = bacc.Bacc(
        "TRN2",
        target_bir_lowering=False,
        debug=False,
        enable_asserts=True,
        num_devices=8,
    )

    xfo_d = nc.dram_tensor("xfo65", [C + 1, N], fp32, kind="ExternalInput").ap()
    xfd_d = nc.dram_tensor("xfd65", [C + 1, N], fp32, kind="ExternalInput").ap()
    gx_d = nc.dram_tensor("gx33", [MC, 33 * NMC], fp32, kind="ExternalInput").ap()
    WwT_d = nc.dram_tensor("W_wT", [C2, C], fp32, kind="ExternalInput").ap()
    xb_d = nc.dram_tensor("xb", [C, N], fp32, kind="ExternalInput").ap()
    out_d = nc.dram_tensor("out", [C, N], fp32, kind="ExternalOutput").ap()

    with tile.TileContext(nc) as tc, ExitStack() as ctx:
        persist = ctx.enter_context(tc.tile_pool(name="persist", bufs=1))
        xfo = persist.tile([C + 1, N], fp32)     # rows 0-63 xf, row 64 = 1.0
        xfoR = persist.tile([C + 1, N], f32r)    # S stationary
        xfd = persist.tile([C + 1, N], fp32)     # rows 0-63 xf, row 64 = -D
        xfdR = persist.tile([C + 1, N], f32r)    # S moving
        gxs = persist.tile([MC, 33 * NMC], fp32)
        gxr = persist.tile([MC, 33 * NMC], f32r)
        WwT_f = persist.tile([C2, C], fp32)
        WwT_r = persist.tile([C2, C], f32r)
        xbt = persist.tile([C, N], fp32)
        ones1 = persist.tile([C2 + 1, C2], fp32)    # row 32 used (lane-aligned w/ d)
        ones1r = persist.tile([C2 + 1, C2], f32r)

        # DMA priority order. Quarter 0 needs: xfo cols 0:4096 (stationary,
        # progressively), xfd cols 0:1024 only (quarter 0 moving), gx head.
        # xfd cols 1024:4096 are for quarters 1-3 (t>40us) -> issued last.
        # Two HWDGE queues: sync carries xfd+gx-head+xb, scalar carries xfo.
        # Converts beyond the first chunks are staged inside the quarter-0
        # loop so they never head-of-line-block the ACT exp stream.
        XFO_CH = [(0, 512), (512, 1024), (1024, 2048), (2048, 3072), (3072, N)]
        XFD_CH = [(0, 512), (512, 1024), (1024, 2048), (2048, 3072), (3072, N)]
        nc.sync.dma_start(xfd[:, 0:512], xfd_d[:, 0:512])
        nc.scalar.dma_start(xfo[:, 0:512], xfo_d[:, 0:512])
        nc.sync.dma_start(gxs[:, 0:132], gx_d[:, 0:132])
        nc.scalar.dma_start(xfo[:, 512:1024], xfo_d[:, 512:1024])
        nc.sync.dma_start(xfd[:, 512:1024], xfd_d[:, 512:1024])
        for a, b in XFO_CH[2:]:
            nc.scalar.dma_start(xfo[:, a:b], xfo_d[:, a:b])
        nc.gpsimd.dma_start(gxs[:, 132:], gx_d[:, 132:])
        nc.gpsimd.dma_start(WwT_f[:], WwT_d)
        nc.sync.dma_start(xbt[:], xb_d)
        for a, b in XFD_CH[2:]:
            nc.sync.dma_start(xfd[:, a:b], xfd_d[:, a:b])
        nc.any.memset(ones1[C2 : C2 + 1, :], 1.0)

        # early converts: just enough for the first chunks of quarter 0
        for a, b in XFO_CH[:2]:
            nc.scalar.activation(xfoR[:, a:b], xfo[:, a:b], AF.Copy)
        for a, b in XFD_CH[:2]:
            nc.vector.tensor_copy(xfdR[:, a:b], xfd[:, a:b])
        nc.vector.tensor_copy(gxr[:, 0:132], gxs[:, 0:132])
        nc.vector.tensor_copy(ones1r[C2 : C2 + 1, :], ones1[C2 : C2 + 1, :])

        def staged_converts(nq, q):
            # remaining f32r converts, spread through quarter 0
            if nq != 0:
                return
            if q == 0:
                nc.vector.tensor_copy(gxr[:, 132:], gxs[:, 132:])
                nc.vector.tensor_copy(WwT_r[:], WwT_f[:])
            elif q in (2, 6, 10):
                a, b = XFO_CH[2 + (q - 2) // 4]
                nc.scalar.activation(xfoR[:, a:b], xfo[:, a:b], AF.Copy)
            elif q in (16, 20, 24):
                a, b = XFD_CH[2 + (q - 16) // 4]
                nc.vector.tensor_copy(xfdR[:, a:b], xfd[:, a:b])

        gxR = gxr[:]
        WwT_R = WwT_r[:]

        s_pool = ctx.enter_context(tc.tile_pool(name="spsum", bufs=3, space="PSUM"))
        y_pool = ctx.enter_context(tc.tile_pool(name="ypsum", bufs=1, space="PSUM"))
        y0 = y_pool.tile([C2 + 1, QW], fp32)     # banks 6-7, partitions 0-32

        e_pool = ctx.enter_context(tc.tile_pool(name="e", bufs=7))
        ysb_pool = ctx.enter_context(tc.tile_pool(name="ysb", bufs=2))
        y1_pool = ctx.enter_context(tc.tile_pool(name="y1", bufs=2))
        r_pool = ctx.enter_context(tc.tile_pool(name="r", bufs=2))
        o_pool = ctx.enter_context(tc.tile_pool(name="osb", bufs=3))

        def tail_part2(n0, r_f, y_sb):
            # previous quarter's normalize + W-projection + residual + store.
            # Emitted mid-way through the NEXT quarter so the r-dependent
            # matmuls never head-of-line-block the PE FIFO. Borrows one
            # s-slot for its PSUM outputs.
            borrow = s_pool.tile([MC, QW], fp32, tag="S")
            rbc = borrow[0:C2, HB:QW]
            z_t = borrow[0:C, 0:HB]
            for h in range(2):
                nc.tensor.matmul(
                    rbc,
                    lhsT=ones1r[C2 : C2 + 1, :],
                    rhs=r_f[:, h * HB : (h + 1) * HB],
                    start=True,
                    stop=True,
                )
                y1 = y1_pool.tile([C2, HB], f32r)
                nc.vector.tensor_mul(y1[:], y_sb[:, h * HB : (h + 1) * HB], rbc)
                nc.tensor.matmul(
                    z_t,
                    lhsT=WwT_R[:],
                    rhs=y1[:],
                    start=True,
                    stop=True,
                )
                o_t = o_pool.tile([C, HB], fp32)
                nc.vector.tensor_add(
                    o_t[:], z_t, xbt[:, n0 + h * HB : n0 + (h + 1) * HB]
                )
                nc.sync.dma_start(
                    out_d[:, n0 + h * HB : n0 + (h + 1) * HB], o_t[:]
                )

        pending = None
        for nq in range(NQ):
            n0 = nq * QW
            for q in range(NMC):
                s_t = s_pool.tile([MC, QW], fp32, tag="S")
                for h in range(2):
                    nc.tensor.matmul(
                        s_t[:, h * HB : (h + 1) * HB],
                        lhsT=xfoR[:, q * MC : (q + 1) * MC],
                        rhs=xfdR[:, n0 + h * HB : n0 + (h + 1) * HB],
                        start=True,
                        stop=True,
                    )
                e_t = e_pool.tile([MC, QW], f32r)
                nc.scalar.activation(e_t[:], s_t[:], AF.Exp)
                for h in range(2):
                    nc.tensor.matmul(
                        y0[:, h * HB : (h + 1) * HB],
                        lhsT=gxR[:, q * 33 : (q + 1) * 33],
                        rhs=e_t[:, h * HB : (h + 1) * HB],
                        start=(q == 0),
                        stop=(q == NMC - 1),
                    )
                staged_converts(nq, q)
                if q == 10 and pending is not None:
                    tail_part2(*pending)
                    pending = None

            # tail part 1: evacuate y0 (numerators + denominator row) to
            # SBUF in ONE copy -- the next quarter's y0 accumulation only
            # waits on this; the reciprocal then runs off the critical
            # path from the SBUF copy. For the last quarter ACT computes
            # r = exp(-ln(d)) instead: ACT is idle after the final exp
            # tile and its ~2us (+2 table swaps) beats the DVE iterative
            # divide's 6.5us on the exposed final tail.
            y_sb33 = ysb_pool.tile([C2 + 1, QW], fp32)
            nc.vector.tensor_copy(y_sb33[:], y0[:])
            y_sb = y_sb33[0:C2, :]
            d_sb = y_sb33[C2 : C2 + 1, :]        # partition 32, lane-aligned
            r_t = r_pool.tile([C2 + 1, QW], f32r, tag="rr")
            r_f = r_t[C2 : C2 + 1, :]
            if nq == NQ - 1:
                ln_t = r_pool.tile([C2 + 1, QW], fp32, tag="ln")
                nc.scalar.activation(ln_t[C2 : C2 + 1, :], d_sb, AF.Ln)
                nc.scalar.activation(
                    r_f, ln_t[C2 : C2 + 1, :], AF.Exp, scale=-1.0
                )
            else:
                with nc.allow_low_precision(reason="1/d f32r; 11-bit ok"):
                    nc.vector.reciprocal(r_f, d_sb)
            pending = (n0, r_f, y_sb)
        tail_part2(*pending)

    nc.compile()
    return nc


def _get_nc():
    if "nc" not in _CACHE:
        _CACHE["nc"] = _build_nc()
    return _CACHE["nc"]


def _run(inputs, trace=False, **kw):
    _ensure_path()
    from concourse.bass_utils import run_bass_kernel_spmd

    nc = _get_nc()
    x = np.ascontiguousarray(np.asarray(inputs["x"], dtype=np.float32))
    g_w = np.asarray(inputs["g_w"], dtype=np.float32)
    g_b = np.asarray(inputs["g_b"], dtype=np.float32)
    W_w = np.asarray(inputs["W_w"], dtype=np.float32)
    W_b = np.asarray(inputs["W_b"], dtype=np.float32)

    WwT = np.ascontiguousarray(W_w.T)                         # [C2, C]
    b_eff = (
        W_w.astype(np.float64) @ g_b.astype(np.float64) + W_b.astype(np.float64)
    ).astype(np.float32).reshape(C, 1)

    B = x.shape[0]
    in_maps = []
    for i in range(B):
        xf = np.ascontiguousarray(x[i].reshape(C, N))
        D = np.einsum("cn,cn->n", xf, xf).astype(np.float32)
        gx = xf.T @ g_w.T                                     # [N, C2]
        gx33 = np.concatenate([gx, np.ones((N, 1), np.float32)], axis=1)
        gx33 = np.ascontiguousarray(
            gx33.reshape(NMC, MC, 33).transpose(1, 0, 2).reshape(MC, 33 * NMC)
        )
        in_maps.append(
            {
                "xfo65": np.ascontiguousarray(
                    np.concatenate([xf, np.ones((1, N), np.float32)], axis=0)
                ),
                "xfd65": np.ascontiguousarray(
                    np.concatenate([xf, -D[None, :]], axis=0)
                ),
                "gx33": gx33,
                "W_wT": WwT,
                "xb": np.ascontiguousarray(xf + b_eff),
            }
        )
    res = run_bass_kernel_spmd(nc, in_maps, list(range(B)), trace=trace, **kw)
    out = np.stack([res.results[i]["out"].reshape(C, 64, 64) for i in range(B)])
    return res, out.astype(np.float32)


def kernel(**inputs):
    _, out = _run(inputs, trace=False)
    return out
